# revision 43
# baseline (speedup 1.0000x reference)
"""Trainium2 Bass kernel for multi-head attention (B=8, N=1024, C=1024, H=16).

Sharding: pure data parallel - one batch element per NeuronCore (8 cores),
no collectives. Host pre-transposes/casts weights and activations to bf16;
all matmuls run bf16 with fp32 PSUM accumulation.

v2 schedule: one uniform software pipeline instead of serial phases.
  - Input DMA is column-sliced so the q0/k0/q1/k1 weight slices land first;
    scores for pair 0 start ~20us earlier than with full-row weight loads.
  - 32 dummy warmup matmuls run during the DMA lead-in so the PE HAM clock
    gate is already at 2.4 GHz when real work arrives.
  - Pair p's PV + normalization run as PE filler inside pair p+1's score
    slots, so ACT (exp) never sees a pair-boundary bubble and the old 22us
    "build all v tiles" hole is gone (v tiles are fillers too).
  - Within each slot, fillers are emitted BEFORE the 4 score matmuls, so
    both score PSUM slots are free by the time the score MMs issue and the
    two heads' K=64 matmuls actually run concurrently on row groups 0/64.
  - proj pre-accumulates one tile during pair 7, rest in the epilogue on
    the freed score PSUM slots, output DMAs overlapped per row tile.
"""

import sys

import numpy as np

if "/opt/trn_rl_repo" not in sys.path:
    sys.path.insert(0, "/opt/trn_rl_repo")

import ml_dtypes

BF16 = ml_dtypes.bfloat16

C = 1024          # model dim
N = 1024          # sequence length
H = 16            # heads
D = 64            # head dim
B = 8             # batch == number of cores
KT = C // 128     # 8 contraction tiles
NT = N // 128     # 8 sequence tiles
SCALE = float(D) ** -0.5

_CACHE = {}
LAST_RESULTS = None
DEBUG_DUMPS = False


def _spread(lst, nslots):
    """Distribute list into nslots chunks, preserving order."""
    out = [[] for _ in range(nslots)]
    n = len(lst)
    for i, x in enumerate(lst):
        out[i * nslots // n].append(x)
    return out


def _build_graph(nc, tc, bass, mybir, has_bias):
    from contextlib import ExitStack

    f32 = mybir.dt.float32
    bf16 = mybir.dt.bfloat16
    Exp = mybir.ActivationFunctionType.Exp

    xT_d = nc.dram_tensor("xT", [C + 1, N], bf16, kind="ExternalInput").ap()
    wq_d = nc.dram_tensor("wqkvT", [C + 1, 3 * C], bf16, kind="ExternalInput").ap()
    wp_d = nc.dram_tensor("wprojT", [C + 1, C], bf16, kind="ExternalInput").ap()
    out_d = nc.dram_tensor("out", [N, C], f32, kind="ExternalOutput").ap()
    if DEBUG_DUMPS:
        dbg_qk = nc.dram_tensor("dbg_qk", [16, 128, N], bf16, kind="ExternalOutput").ap()
        dbg_vv = nc.dram_tensor("dbg_vv", [NT, 128, H * 65], bf16, kind="ExternalOutput").ap()
        dbg_es = nc.dram_tensor("dbg_es", [NT, 2, 128, N], bf16, kind="ExternalOutput").ap()
        dbg_ot = nc.dram_tensor("dbg_ot", [KT, 128, N], bf16, kind="ExternalOutput").ap()

    with ExitStack() as ctx:
        persist = ctx.enter_context(tc.tile_pool(name="persist", bufs=1))
        qkp = ctx.enter_context(tc.tile_pool(name="qkp", bufs=5))
        expp = ctx.enter_context(tc.tile_pool(name="expp", bufs=23))
        small = ctx.enter_context(tc.tile_pool(name="small", bufs=4))
        outp = ctx.enter_context(tc.tile_pool(name="outp", bufs=2))
        # PSUM budget 8 banks: pmm 2x[128,512] (2) + pss 2x[128,1024] (4)
        # + po 2x[128,512]-sized (2).
        pmm = ctx.enter_context(tc.tile_pool(name="pmm", bufs=2, space="PSUM"))
        pss = ctx.enter_context(tc.tile_pool(name="pss", bufs=2, space="PSUM"))
        po = ctx.enter_context(tc.tile_pool(name="po", bufs=2, space="PSUM"))
        drp = ctx.enter_context(tc.tile_pool(name="drp", bufs=2, space="DRAM"))

        # ---- persistent SBUF tensors ----
        xt = [persist.tile([128, N], bf16, tag=f"xt{i}", name=f"xt{i}") for i in range(KT)]
        wq = [persist.tile([128, 3 * C], bf16, tag=f"wq{i}", name=f"wq{i}") for i in range(KT)]
        wp = [persist.tile([128, C], bf16, tag=f"wp{i}", name=f"wp{i}") for i in range(KT)]
        vv = [persist.tile([128, H * 65], bf16, tag=f"vv{i}", name=f"vv{i}") for i in range(NT)]
        ot = [persist.tile([128, N], bf16, tag=f"ot{i}", name=f"ot{i}") for i in range(KT)]
        scrw = persist.tile([128, 512], bf16, tag="scrw", name="scrw")
        if has_bias:
            xones = persist.tile([1, N], bf16, tag="xones", name="xones")
            wqb = persist.tile([1, 3 * C], bf16, tag="wqb", name="wqb")
            wpb = persist.tile([1, C], bf16, tag="wpb", name="wpb")

        # ---- PE warmup: dummy matmuls during the DMA lead-in keep the HAM
        # clock gate at 2.4GHz so the first real matmuls aren't half-rate.
        nc.vector.memset(scrw[:], 0.0)
        pwarm = po.tile([128, 512], f32, tag="o", name="pwarm")
        for _ in range(16):
            nc.tensor.matmul(pwarm[:], scrw[:, 0:128], scrw[:], start=True, stop=True)
        wdrain = small.tile([1, 16], f32, tag="wdrain", name="wdrain")
        nc.vector.tensor_copy(wdrain[:], pwarm[0:1, 0:16])

        # preload the Exp activation table during the DMA phase.
        warm = small.tile([1, 16], f32, tag="warm", name="warm")
        nc.vector.memset(warm[:], 0.0)
        nc.scalar.activation(warm[:], warm[:], Exp, scale=1.0)

        # ---- input DMAs, column-sliced by first use ----
        def eng(i):
            return nc.sync if i % 2 == 0 else nc.gpsimd

        def wq_slice(kt, c0, c1, e):
            e.dma_start(wq[kt][:, c0:c1], wq_d[kt * 128:(kt + 1) * 128, c0:c1])

        # critical prefix: xt + q0,k0,q1,k1 column slices, kt-major
        USE_SLICED = True
        if USE_SLICED:
            for kt in range(KT):
                e = eng(kt)
                e.dma_start(xt[kt][:], xT_d[kt * 128:(kt + 1) * 128, :])
                for t in (0, 8):
                    wq_slice(kt, t * 128, (t + 1) * 128, e)
            # q1/k1 (first pair-0 builders in the static schedule), then v
            for kt in range(KT):
                e = eng(kt)
                for t in (1, 9):
                    wq_slice(kt, t * 128, (t + 1) * 128, e)
            for kt in range(KT):
                wq_slice(kt, 2 * C, 3 * C, eng(kt + 1))
            # q rest, k rest
            for kt in range(KT):
                wq_slice(kt, 2 * 128, C, eng(kt))
                wq_slice(kt, C + 2 * 128, 2 * C, eng(kt))
        else:
            for kt in range(KT):
                e = eng(kt)
                e.dma_start(xt[kt][:], xT_d[kt * 128:(kt + 1) * 128, :])
                e.dma_start(wq[kt][:], wq_d[kt * 128:(kt + 1) * 128, :])
        # proj weights last
        for kt in range(KT):
            eng(kt + 1).dma_start(wp[kt][:], wp_d[kt * 128:(kt + 1) * 128, :])
        if has_bias:
            nc.sync.dma_start(xones[:], xT_d[C:C + 1, :])
            nc.sync.dma_start(wqb[:], wq_d[C:C + 1, :])
            nc.sync.dma_start(wpb[:], wp_d[C:C + 1, :])

        qk = {}   # qk-tile index (0..7 q, 8..15 k) -> sbuf tile
        es = {}   # (pair, j) -> (eA, eB)

        def qk_builder(j_tile, pool, tag):
            """Incremental qk tile: qkT[o, n] = w_qkvT[:, o].T @ xT."""
            t = qkp.tile([128, N], bf16, tag="qk", name=f"qk{j_tile}")
            ph = [pool.tile([128, 512], f32, tag=tag, name=f"ps_qk{j_tile}_{x}")
                  for x in range(2)]

            def step(kt):
                for half in range(2):
                    sl = bass.ts(half, 512)
                    nc.tensor.matmul(
                        ph[half][:], wq[kt][:, j_tile * 128:(j_tile + 1) * 128],
                        xt[kt][:, sl],
                        start=(kt == 0), stop=(kt == KT - 1 and not has_bias))
                    if has_bias and kt == KT - 1:
                        nc.tensor.matmul(
                            ph[half][:], wqb[:, j_tile * 128:(j_tile + 1) * 128],
                            xones[:, sl], start=False, stop=True)

            def finish():
                for half in range(2):
                    nc.vector.tensor_copy(t[:, bass.ts(half, 512)], ph[half][:])
                qk[j_tile] = t
                if DEBUG_DUMPS:
                    nc.sync.dma_start(dbg_qk[j_tile], t[:])

            return step, finish

        def v_builder(nt, pool, tag):
            """Incremental v tile: v[n, o] = xT[:, n].T @ w_qkvT[:, 2C:].
            Stored with stride-65 head blocks; col 64 = ones (rowsum trick)."""
            dst = vv[nt][:].rearrange("p (h w) -> p h w", w=65)
            phs = [pool.tile([128, 512], f32, tag=tag, name=f"ps_v{nt}_{x}")
                   for x in range(2)]

            def step(kt):
                for half in range(2):
                    sl = bass.ds(2 * C + half * 512, 512)
                    nc.tensor.matmul(
                        phs[half][:], xt[kt][:, nt * 128:(nt + 1) * 128],
                        wq[kt][:, sl],
                        start=(kt == 0), stop=(kt == KT - 1 and not has_bias))
                    if has_bias and kt == KT - 1:
                        nc.tensor.matmul(
                            phs[half][:], xones[:, nt * 128:(nt + 1) * 128],
                            wqb[:, sl], start=False, stop=True)

            def finish():
                for half in range(2):
                    nc.vector.tensor_copy(
                        dst[:, half * 8:(half + 1) * 8, 0:64],
                        phs[half][:].rearrange("p (h w) -> p h w", w=64))
                nc.gpsimd.memset(dst[:, :, 64:65], 1.0)
                if DEBUG_DUMPS:
                    nc.sync.dma_start(dbg_vv[nt], vv[nt][:])

            return step, finish

        def builder_units(mk):
            """Expand a builder into a list of emit-closures (8 steps + finish)."""
            step, fin = mk()
            return [(lambda s=step, k=kt: s(k)) for kt in range(KT)] + [fin]

        def scores(p, j):
            """Emit pair-p scores for nk-tile j + 2 exps. Each head's matmul
            is split into two M=64 halves so the 4 matmuls per nq-half occupy
            DISJOINT 64x64 quadrant sets (rows 0/64 x cols 0/64) and run
            4-way concurrent on the PE (span ~= one matmul). Emitted LAST in
            each slot so both pss slots are already free when they issue."""
            qA = qk[p][0:64, :]
            kA = qk[8 + p][0:64, :]
            qB = qk[p][64:128, :]
            kB = qk[8 + p][64:128, :]
            jsl = slice(j * 128, (j + 1) * 128)
            psA = pss.tile([128, N], f32, tag="s", name=f"s{p}_{j}a")
            psB = pss.tile([128, N], f32, tag="s", name=f"s{p}_{j}b")
            nc.tensor.matmul(psA[:, 0:512], kA[:, jsl], qA[:, 0:512],
                             start=True, stop=True)
            nc.tensor.matmul(psB[:, 0:512], kB[:, jsl], qB[:, 0:512],
                             start=True, stop=True)
            nc.tensor.matmul(psA[:, 512:1024], kA[:, jsl], qA[:, 512:1024],
                             start=True, stop=True)
            eA = expp.tile([128, N], bf16, tag="es", name=f"e{p}_{j}a")
            nc.scalar.activation(eA[:], psA[:], Exp, scale=SCALE)
            nc.tensor.matmul(psB[:, 512:1024], kB[:, jsl], qB[:, 512:1024],
                             start=True, stop=True)
            eB = expp.tile([128, N], bf16, tag="es", name=f"e{p}_{j}b")
            nc.scalar.activation(eB[:], psB[:], Exp, scale=SCALE)
            es[(p, j)] = (eA, eB)
            if DEBUG_DUMPS and p == 0:
                nc.sync.dma_start(dbg_es[j, 0], eA[:])
                nc.sync.dma_start(dbg_es[j, 1], eB[:])

        def po_tiles(h, pool=None, tag=None):
            pool = pool or po
            tag = tag or "o"
            return [pool.tile([65, 512], f32, tag=tag, name=f"pso{h}_{x}")
                    for x in range(2)]

        def pv_step(h, psos, j, e):
            """One nk-tile of [O'^T ; rowsum] accumulation (both nq halves)."""
            for half in range(2):
                esl = bass.ts(half, 512)
                nc.tensor.matmul(
                    psos[half][:], vv[j][:, h * 65:(h + 1) * 65], e[:, esl],
                    start=(j == 0), stop=(j == NT - 1))

        def norm(h, psos):
            """Normalize O'^T by its rowsum into ot (DVE + DRAM-bounce
            partition broadcast; see baseline docstring)."""
            off = (h % 2) * 64
            for half in range(2):
                sl = bass.ts(half, 512)
                pso = psos[half]
                o_sb = small.tile([64, 512], bf16, tag="osb2", name=f"o_sb{h}_{half}")
                nc.vector.tensor_copy(o_sb[:], pso[0:64, :])
                srow = small.tile([1, 512], f32, tag="srow", name=f"srow{h}_{half}")
                nc.vector.tensor_copy(srow[:], pso[64:65, :])
                r1 = small.tile([1, 512], f32, tag="rc", name=f"rc{h}_{half}")
                nc.vector.reciprocal_approx_fast(out=r1[:], in_=srow[:])
                r1b = small.tile([1, 512], bf16, tag="rcb", name=f"rcb{h}_{half}")
                nc.vector.tensor_copy(r1b[:], r1[:])
                scr = drp.tile([1, 512], bf16, tag="scr", name=f"scr{h}_{half}")
                nc.gpsimd.dma_start(scr[:], r1b[:])
                s = scr[:]
                src_b = bass.AP(tensor=s.tensor, offset=s.offset,
                                ap=[[0, 64]] + list(s.ap[1:]))
                rbc = small.tile([64, 512], bf16, tag="rbc", name=f"rbc{h}_{half}")
                nc.gpsimd.dma_start(rbc[:], src_b)
                nc.vector.tensor_mul(ot[h // 2][off:off + 64, sl], o_sb[:], rbc[:])

        ones64 = persist.tile([1, 64], bf16, tag="ones64", name="ones64")
        nc.gpsimd.memset(ones64[:], 1.0)

        def norm_tail(h, psos, bpool, btag):
            """Tail-pair norm: partition-broadcast via a K=1 PE matmul instead
            of the DRAM bounce — shorter critical chain for the last heads.
            The o_sb/srow copies release the psos slots BEFORE pbc allocates
            from the same pool (deadlock-free); the final mul reads the
            broadcast directly from PSUM (one PSUM operand is legal)."""
            off = (h % 2) * 64
            for half in range(2):
                sl = bass.ts(half, 512)
                pso = psos[half]
                o_sb = small.tile([64, 512], bf16, tag="osb2", name=f"t_osb{h}_{half}")
                nc.vector.tensor_copy(o_sb[:], pso[0:64, :])
                srow = small.tile([1, 512], f32, tag="srow", name=f"tsrow{h}_{half}")
                nc.vector.tensor_copy(srow[:], pso[64:65, :])
                r1 = small.tile([1, 512], f32, tag="rc", name=f"trc{h}_{half}")
                nc.vector.reciprocal_approx_fast(out=r1[:], in_=srow[:])
                r1b = small.tile([1, 512], bf16, tag="rcb", name=f"trcb{h}_{half}")
                nc.scalar.copy(r1b[:], r1[:])
                pbc = bpool.tile([64, 512], f32, tag=btag, name=f"tpbc{h}_{half}")
                nc.tensor.matmul(pbc[:], ones64[:], r1b[:], start=True, stop=True)
                nc.vector.tensor_mul(ot[h // 2][off:off + 64, sl], o_sb[:], pbc[:])

        def pv_units(pm, tail=False):
            """PV + norm of pair pm as a unit list (consumed in pair pm+1).
            tail=True: PV-B runs on pmm (concurrent with PV-A on po, no slot
            serialization) and norms use the PE-broadcast variant."""
            st = {}
            units = []

            def mkA():
                st['A'] = po_tiles(2 * pm)

            def mkB():
                if tail:
                    st['B'] = po_tiles(2 * pm + 1, pss, "s")
                else:
                    st['B'] = po_tiles(2 * pm + 1)

            for j in range(NT):
                def uA(j=j):
                    if 'A' not in st:
                        mkA()
                    pv_step(2 * pm, st['A'], j, es[(pm, j)][0])
                units.append(uA)
            if tail:
                units.append(lambda: norm_tail(2 * pm, st['A'], po, "o"))
            else:
                units.append(lambda: norm(2 * pm, st['A']))
            for j in range(NT):
                def uB(j=j):
                    if 'B' not in st:
                        mkB()
                    pv_step(2 * pm + 1, st['B'], j, es[(pm, j)][1])
                units.append(uB)
            if tail:
                units.append(lambda: norm_tail(2 * pm + 1, st['B'], pss, "s"))
            else:
                units.append(lambda: norm(2 * pm + 1, st['B']))
            return units

        # ---- prologue: q0 (pmm), k0 (pss) only, paced by DMA arrivals —
        # q1/k1 become pair-0 fillers so the first scores land sooner.
        b_q0s, b_q0f = qk_builder(0, pmm, "mm")
        b_k0s, b_k0f = qk_builder(8, pss, "s")
        for kt in range(KT):
            b_q0s(kt)
            b_k0s(kt)
        b_q0f()
        b_k0f()

        # ---- builder assignment per pair (fillers) ----
        def QK(j, pool, tag):
            return lambda: builder_units(lambda: qk_builder(j, pool, tag))

        def VB(nt, pool, tag):
            return lambda: builder_units(lambda: v_builder(nt, pool, tag))

        # NOTE: every v builder must be EMITTED in pair 0 — Tile's dependency
        # tracking is program-order-based, so a PV read of vv[j] emitted
        # before the v builder's writes would silently miss the dependency.
        builders_by_pair = [
            [QK(1, po, "o"), QK(9, po, "o"), VB(0, pmm, "mm"), VB(1, po, "o"),
             VB(2, pmm, "mm"), VB(3, po, "o"), VB(4, pmm, "mm"),
             VB(5, po, "o"), VB(6, pmm, "mm"), VB(7, po, "o")],
            [QK(2, pmm, "mm"), QK(10, pmm, "mm")],
            [QK(3, pmm, "mm"), QK(11, pmm, "mm")],
            [QK(4, pmm, "mm"), QK(12, pmm, "mm")],
            [QK(5, pmm, "mm"), QK(13, pmm, "mm")],
            [QK(6, pmm, "mm"), QK(14, pmm, "mm")],
            [QK(7, pmm, "mm"), QK(15, pmm, "mm")],
            [],  # pair 7: proj pre-accumulation, set up below
        ]

        def proj_tile(nt, pool):
            """proj output row-tile nt: final[nq, co] = sum_kt ot[kt].T @ wp[kt].
            pss pool: one [128,1024] tile (both banks); else two [128,512]."""
            st = {}
            ntsl = slice(nt * 128, (nt + 1) * 128)

            def step(kt):
                if 'ph' not in st:
                    if pool is pss:
                        t = pss.tile([128, N], f32, tag="s", name=f"ps_pj{nt}")
                        st['ph'] = [t[:, 0:512], t[:, 512:1024]]
                    else:
                        tg = "mm" if pool is pmm else "o"
                        st['ph'] = [pool.tile([128, 512], f32, tag=tg,
                                              name=f"ps_pj{nt}_{x}")[:]
                                    for x in range(2)]
                for half in range(2):
                    sl = bass.ts(half, 512)
                    nc.tensor.matmul(
                        st['ph'][half], ot[kt][:, ntsl], wp[kt][:, sl],
                        start=(kt == 0), stop=(kt == KT - 1 and not has_bias))
                    if has_bias and kt == KT - 1:
                        nc.tensor.matmul(
                            st['ph'][half], xones[:, ntsl], wpb[:, sl],
                            start=False, stop=True)

            def finish():
                osb = outp.tile([128, N], f32, tag="osb", name=f"osb{nt}")
                for half in range(2):
                    nc.vector.tensor_copy(osb[:, bass.ts(half, 512)], st['ph'][half])
                nc.sync.dma_start(out_d[ntsl, :], osb[:])

            return step, finish

        pj1 = proj_tile(1, pmm)

        # ---- main pipeline: 8 pairs x 8 slots ----
        for p in range(8):
            # expand builder units for this pair
            units = []
            for mk in builders_by_pair[p]:
                units.extend(mk())
            if p == 7:
                # proj pre-accumulation on pmm (free: no builders this pair),
                # kt<=5 only: ot[6] writes (norm of pair 6) are emitted inside
                # THIS pair's pv units — a kt=6 read emitted here would
                # precede them (missed dependency).
                units.extend([(lambda k=kt: pj1[0](k)) for kt in range(6)])
            pvs = pv_units(p - 1) if p > 0 else []
            u_sched = _spread(units, NT) if units else [[] for _ in range(NT)]
            pv_sched = _spread(pvs, NT) if pvs else [[] for _ in range(NT)]
            for j in range(NT):
                if p == 0:
                    # pair 0: scores first — no pv fillers exist and the
                    # builder units are DMA-gated; emitting scores last would
                    # queue them behind ~20 priority-earlier filler MMs and
                    # delay the very first exp by ~5us.
                    scores(p, j)
                    for u in u_sched[j]:
                        u()
                    continue
                if p == 7:
                    # pair 7: ACT is the constraint on when the last exp
                    # lands (PE has slack — no builders), and the whole
                    # epilogue chains off it. Scores first.
                    scores(p, j)
                    for u in pv_sched[j]:
                        u()
                    for u in u_sched[j]:
                        u()
                    continue
                for u in pv_sched[j]:
                    u()
                for u in u_sched[j]:
                    u()
                scores(p, j)

        # ---- epilogue ----
        # PV(7) with PV-A on po, PV-B on pmm (concurrent), PE-broadcast norms.
        for u in pv_units(7, tail=True):
            u()
        if DEBUG_DUMPS:
            for kt in range(KT):
                nc.sync.dma_start(dbg_ot[kt], ot[kt][:])

        # kt=6 step for the pre-accumulated tile (norm(6,B) now emitted)
        pj1[0](6)
        # pj2 rides po the moment norm(7,A)'s copies release it — overlaps
        # with norm(7,B)'s chain on DVE.
        pj2 = proj_tile(2, po)
        for kt in range(7):
            pj2[0](kt)
        # close the pre-accumulated tiles (kt=7 is after norm(7,B) above)
        for pj in (pj1, pj2):
            pj[0](7)
            pj[1]()
        # remaining tiles cascade through the freed pools
        for nt, pool in ((5, pss), (6, pss), (3, pmm), (4, po), (7, pss), (0, pss)):
            s, f = proj_tile(nt, pool)
            for kt in range(KT):
                s(kt)
            f()


def _get_compiled(has_bias):
    key = ("nc", has_bias)
    if key in _CACHE:
        return _CACHE[key]
    import concourse.bass as bass
    import concourse.mybir as mybir
    from concourse import bacc, tile

    nc = bacc.Bacc("TRN2", target_bir_lowering=False, debug=False, num_devices=B)
    with tile.TileContext(nc) as tc:
        _build_graph(nc, tc, bass, mybir, has_bias)
    nc.compile()
    _CACHE[key] = nc
    return nc


def _in_maps(x, w_qkv, b_qkv, w_proj, b_proj):
    xT = np.ascontiguousarray(np.transpose(np.asarray(x, np.float32), (0, 2, 1))).astype(BF16)
    ones = np.ones((1, N), BF16)
    wq = np.concatenate([np.asarray(w_qkv, np.float32).T,
                         np.asarray(b_qkv, np.float32)[None, :]], 0).astype(BF16)
    wp = np.concatenate([np.asarray(w_proj, np.float32).T,
                         np.asarray(b_proj, np.float32)[None, :]], 0).astype(BF16)
    wq = np.ascontiguousarray(wq)
    wp = np.ascontiguousarray(wp)
    return [
        {"xT": np.ascontiguousarray(np.concatenate([xT[b], ones], 0)),
         "wqkvT": wq, "wprojT": wp}
        for b in range(B)
    ]


def _ensure_ntff_hook():
    """The agent image's `antenv` lacks `axon_hooks`; provide the registry
    module + ctypes hook so neuron-profile NTFF capture works when tracing."""
    import importlib
    import types

    try:
        importlib.import_module("antenv.axon_hooks")
        return
    except ImportError:
        pass
    mod = types.ModuleType("antenv.axon_hooks")
    mod._hook = None

    def set_axon_ntff_profile_hook(h):
        mod._hook = h

    def get_axon_ntff_profile_hook():
        return mod._hook

    mod.set_axon_ntff_profile_hook = set_axon_ntff_profile_hook
    mod.get_axon_ntff_profile_hook = get_axon_ntff_profile_hook
    import antenv

    antenv.axon_hooks = mod
    sys.modules["antenv.axon_hooks"] = mod
    try:
        from trn_agent_boot.trn_boot import _ntff_profile_via_ctypes

        hook = _ntff_profile_via_ctypes("/opt/axon/libaxon_pjrt.so")
        if hook is not None:
            mod._hook = hook
    except Exception:
        pass


def kernel(x, w_qkv, b_qkv, w_proj, b_proj):
    global LAST_RESULTS
    import os

    if os.environ.get("BASS_TRACE"):
        _ensure_ntff_hook()
    from concourse.bass_utils import run_bass_kernel_spmd

    has_bias = bool(np.any(np.asarray(b_qkv)) or np.any(np.asarray(b_proj)))
    nc = _get_compiled(has_bias)
    maps = _in_maps(x, w_qkv, b_qkv, w_proj, b_proj)
    res = run_bass_kernel_spmd(nc, maps, core_ids=list(range(B)))
    LAST_RESULTS = res
    return np.stack([res.results[b]["out"] for b in range(B)]).astype(np.float32)


# revision 44
# speedup vs baseline: 1.0007x; 1.0007x over previous
"""Trainium2 Bass kernel for multi-head attention (B=8, N=1024, C=1024, H=16).

Sharding: pure data parallel - one batch element per NeuronCore (8 cores),
no collectives. Host pre-transposes/casts weights and activations to bf16;
all matmuls run bf16 with fp32 PSUM accumulation.

v2 schedule: one uniform software pipeline instead of serial phases.
  - Input DMA is column-sliced so the q0/k0/q1/k1 weight slices land first;
    scores for pair 0 start ~20us earlier than with full-row weight loads.
  - 32 dummy warmup matmuls run during the DMA lead-in so the PE HAM clock
    gate is already at 2.4 GHz when real work arrives.
  - Pair p's PV + normalization run as PE filler inside pair p+1's score
    slots, so ACT (exp) never sees a pair-boundary bubble and the old 22us
    "build all v tiles" hole is gone (v tiles are fillers too).
  - Within each slot, fillers are emitted BEFORE the 4 score matmuls, so
    both score PSUM slots are free by the time the score MMs issue and the
    two heads' K=64 matmuls actually run concurrently on row groups 0/64.
  - proj pre-accumulates one tile during pair 7, rest in the epilogue on
    the freed score PSUM slots, output DMAs overlapped per row tile.
"""

import sys

import numpy as np

if "/opt/trn_rl_repo" not in sys.path:
    sys.path.insert(0, "/opt/trn_rl_repo")

import ml_dtypes

BF16 = ml_dtypes.bfloat16

C = 1024          # model dim
N = 1024          # sequence length
H = 16            # heads
D = 64            # head dim
B = 8             # batch == number of cores
KT = C // 128     # 8 contraction tiles
NT = N // 128     # 8 sequence tiles
SCALE = float(D) ** -0.5

_CACHE = {}
LAST_RESULTS = None
DEBUG_DUMPS = False


def _spread(lst, nslots):
    """Distribute list into nslots chunks, preserving order."""
    out = [[] for _ in range(nslots)]
    n = len(lst)
    for i, x in enumerate(lst):
        out[i * nslots // n].append(x)
    return out


def _build_graph(nc, tc, bass, mybir, has_bias):
    from contextlib import ExitStack

    f32 = mybir.dt.float32
    bf16 = mybir.dt.bfloat16
    Exp = mybir.ActivationFunctionType.Exp

    xT_d = nc.dram_tensor("xT", [C + 1, N], bf16, kind="ExternalInput").ap()
    wq_d = nc.dram_tensor("wqkvT", [C + 1, 3 * C], bf16, kind="ExternalInput").ap()
    wp_d = nc.dram_tensor("wprojT", [C + 1, C], bf16, kind="ExternalInput").ap()
    out_d = nc.dram_tensor("out", [N, C], f32, kind="ExternalOutput").ap()
    if DEBUG_DUMPS:
        dbg_qk = nc.dram_tensor("dbg_qk", [16, 128, N], bf16, kind="ExternalOutput").ap()
        dbg_vv = nc.dram_tensor("dbg_vv", [NT, 128, H * 65], bf16, kind="ExternalOutput").ap()
        dbg_es = nc.dram_tensor("dbg_es", [NT, 2, 128, N], bf16, kind="ExternalOutput").ap()
        dbg_ot = nc.dram_tensor("dbg_ot", [KT, 128, N], bf16, kind="ExternalOutput").ap()

    with ExitStack() as ctx:
        persist = ctx.enter_context(tc.tile_pool(name="persist", bufs=1))
        qkp = ctx.enter_context(tc.tile_pool(name="qkp", bufs=5))
        expp = ctx.enter_context(tc.tile_pool(name="expp", bufs=23))
        small = ctx.enter_context(tc.tile_pool(name="small", bufs=4))
        outp = ctx.enter_context(tc.tile_pool(name="outp", bufs=2))
        # PSUM budget 8 banks: pmm 2x[128,512] (2) + pss 2x[128,1024] (4)
        # + po 2x[128,512]-sized (2).
        pmm = ctx.enter_context(tc.tile_pool(name="pmm", bufs=2, space="PSUM"))
        pss = ctx.enter_context(tc.tile_pool(name="pss", bufs=2, space="PSUM"))
        po = ctx.enter_context(tc.tile_pool(name="po", bufs=2, space="PSUM"))
        drp = ctx.enter_context(tc.tile_pool(name="drp", bufs=2, space="DRAM"))

        # ---- persistent SBUF tensors ----
        xt = [persist.tile([128, N], bf16, tag=f"xt{i}", name=f"xt{i}") for i in range(KT)]
        wq = [persist.tile([128, 3 * C], bf16, tag=f"wq{i}", name=f"wq{i}") for i in range(KT)]
        wp = [persist.tile([128, C], bf16, tag=f"wp{i}", name=f"wp{i}") for i in range(KT)]
        vv = [persist.tile([128, H * 65], bf16, tag=f"vv{i}", name=f"vv{i}") for i in range(NT)]
        ot = [persist.tile([128, N], bf16, tag=f"ot{i}", name=f"ot{i}") for i in range(KT)]
        scrw = persist.tile([128, 512], bf16, tag="scrw", name="scrw")
        if has_bias:
            xones = persist.tile([1, N], bf16, tag="xones", name="xones")
            wqb = persist.tile([1, 3 * C], bf16, tag="wqb", name="wqb")
            wpb = persist.tile([1, C], bf16, tag="wpb", name="wpb")

        # ---- PE warmup: dummy matmuls during the DMA lead-in keep the HAM
        # clock gate at 2.4GHz so the first real matmuls aren't half-rate.
        nc.vector.memset(scrw[:], 0.0)
        pwarm = po.tile([128, 512], f32, tag="o", name="pwarm")
        for _ in range(16):
            nc.tensor.matmul(pwarm[:], scrw[:, 0:128], scrw[:], start=True, stop=True)
        wdrain = small.tile([1, 16], f32, tag="wdrain", name="wdrain")
        nc.vector.tensor_copy(wdrain[:], pwarm[0:1, 0:16])

        # preload the Exp activation table during the DMA phase.
        warm = small.tile([1, 16], f32, tag="warm", name="warm")
        nc.vector.memset(warm[:], 0.0)
        nc.scalar.activation(warm[:], warm[:], Exp, scale=1.0)

        # ---- input DMAs, column-sliced by first use ----
        def eng(i):
            return nc.sync if i % 2 == 0 else nc.gpsimd

        def wq_slice(kt, c0, c1, e):
            e.dma_start(wq[kt][:, c0:c1], wq_d[kt * 128:(kt + 1) * 128, c0:c1])

        # critical prefix: xt + q0,k0,q1,k1 column slices, kt-major
        USE_SLICED = True
        if USE_SLICED:
            for kt in range(KT):
                e = eng(kt)
                e.dma_start(xt[kt][:], xT_d[kt * 128:(kt + 1) * 128, :])
                for t in (0, 8):
                    wq_slice(kt, t * 128, (t + 1) * 128, e)
            # q1/k1 (first pair-0 builders in the static schedule), then v
            for kt in range(KT):
                e = eng(kt)
                for t in (1, 9):
                    wq_slice(kt, t * 128, (t + 1) * 128, e)
            for kt in range(KT):
                wq_slice(kt, 2 * C, 3 * C, eng(kt + 1))
            # q rest, k rest
            for kt in range(KT):
                wq_slice(kt, 2 * 128, C, eng(kt))
                wq_slice(kt, C + 2 * 128, 2 * C, eng(kt))
        else:
            for kt in range(KT):
                e = eng(kt)
                e.dma_start(xt[kt][:], xT_d[kt * 128:(kt + 1) * 128, :])
                e.dma_start(wq[kt][:], wq_d[kt * 128:(kt + 1) * 128, :])
        # proj weights last
        for kt in range(KT):
            eng(kt + 1).dma_start(wp[kt][:], wp_d[kt * 128:(kt + 1) * 128, :])
        if has_bias:
            nc.sync.dma_start(xones[:], xT_d[C:C + 1, :])
            nc.sync.dma_start(wqb[:], wq_d[C:C + 1, :])
            nc.sync.dma_start(wpb[:], wp_d[C:C + 1, :])

        qk = {}   # qk-tile index (0..7 q, 8..15 k) -> sbuf tile
        es = {}   # (pair, j) -> (eA, eB)

        def qk_builder(j_tile, pool, tag):
            """Incremental qk tile: qkT[o, n] = w_qkvT[:, o].T @ xT."""
            t = qkp.tile([128, N], bf16, tag="qk", name=f"qk{j_tile}")
            ph = [pool.tile([128, 512], f32, tag=tag, name=f"ps_qk{j_tile}_{x}")
                  for x in range(2)]

            def step(kt):
                for half in range(2):
                    sl = bass.ts(half, 512)
                    nc.tensor.matmul(
                        ph[half][:], wq[kt][:, j_tile * 128:(j_tile + 1) * 128],
                        xt[kt][:, sl],
                        start=(kt == 0), stop=(kt == KT - 1 and not has_bias))
                    if has_bias and kt == KT - 1:
                        nc.tensor.matmul(
                            ph[half][:], wqb[:, j_tile * 128:(j_tile + 1) * 128],
                            xones[:, sl], start=False, stop=True)

            def finish():
                for half in range(2):
                    nc.vector.tensor_copy(t[:, bass.ts(half, 512)], ph[half][:])
                qk[j_tile] = t
                if DEBUG_DUMPS:
                    nc.sync.dma_start(dbg_qk[j_tile], t[:])

            return step, finish

        def v_builder(nt, pool, tag):
            """Incremental v tile: v[n, o] = xT[:, n].T @ w_qkvT[:, 2C:].
            Stored with stride-65 head blocks; col 64 = ones (rowsum trick)."""
            dst = vv[nt][:].rearrange("p (h w) -> p h w", w=65)
            phs = [pool.tile([128, 512], f32, tag=tag, name=f"ps_v{nt}_{x}")
                   for x in range(2)]

            def step(kt):
                for half in range(2):
                    sl = bass.ds(2 * C + half * 512, 512)
                    nc.tensor.matmul(
                        phs[half][:], xt[kt][:, nt * 128:(nt + 1) * 128],
                        wq[kt][:, sl],
                        start=(kt == 0), stop=(kt == KT - 1 and not has_bias))
                    if has_bias and kt == KT - 1:
                        nc.tensor.matmul(
                            phs[half][:], xones[:, nt * 128:(nt + 1) * 128],
                            wqb[:, sl], start=False, stop=True)

            def finish():
                for half in range(2):
                    nc.vector.tensor_copy(
                        dst[:, half * 8:(half + 1) * 8, 0:64],
                        phs[half][:].rearrange("p (h w) -> p h w", w=64))
                nc.gpsimd.memset(dst[:, :, 64:65], 1.0)
                if DEBUG_DUMPS:
                    nc.sync.dma_start(dbg_vv[nt], vv[nt][:])

            return step, finish

        def builder_units(mk):
            """Expand a builder into a list of emit-closures (8 steps + finish)."""
            step, fin = mk()
            return [(lambda s=step, k=kt: s(k)) for kt in range(KT)] + [fin]

        def scores(p, j):
            """Emit pair-p scores for nk-tile j + 2 exps. Each head's matmul
            is split into two M=64 halves so the 4 matmuls per nq-half occupy
            DISJOINT 64x64 quadrant sets (rows 0/64 x cols 0/64) and run
            4-way concurrent on the PE (span ~= one matmul). Emitted LAST in
            each slot so both pss slots are already free when they issue."""
            qA = qk[p][0:64, :]
            kA = qk[8 + p][0:64, :]
            qB = qk[p][64:128, :]
            kB = qk[8 + p][64:128, :]
            jsl = slice(j * 128, (j + 1) * 128)
            psA = pss.tile([128, N], f32, tag="s", name=f"s{p}_{j}a")
            psB = pss.tile([128, N], f32, tag="s", name=f"s{p}_{j}b")
            nc.tensor.matmul(psA[:, 0:512], kA[:, jsl], qA[:, 0:512],
                             start=True, stop=True)
            nc.tensor.matmul(psB[:, 0:512], kB[:, jsl], qB[:, 0:512],
                             start=True, stop=True)
            nc.tensor.matmul(psA[:, 512:1024], kA[:, jsl], qA[:, 512:1024],
                             start=True, stop=True)
            eA = expp.tile([128, N], bf16, tag="es", name=f"e{p}_{j}a")
            nc.scalar.activation(eA[:], psA[:], Exp, scale=SCALE)
            nc.tensor.matmul(psB[:, 512:1024], kB[:, jsl], qB[:, 512:1024],
                             start=True, stop=True)
            eB = expp.tile([128, N], bf16, tag="es", name=f"e{p}_{j}b")
            nc.scalar.activation(eB[:], psB[:], Exp, scale=SCALE)
            es[(p, j)] = (eA, eB)
            if DEBUG_DUMPS and p == 0:
                nc.sync.dma_start(dbg_es[j, 0], eA[:])
                nc.sync.dma_start(dbg_es[j, 1], eB[:])

        def po_tiles(h, pool=None, tag=None):
            pool = pool or po
            tag = tag or "o"
            return [pool.tile([65, 512], f32, tag=tag, name=f"pso{h}_{x}")
                    for x in range(2)]

        def pv_step(h, psos, j, e):
            """One nk-tile of [O'^T ; rowsum] accumulation (both nq halves)."""
            for half in range(2):
                esl = bass.ts(half, 512)
                nc.tensor.matmul(
                    psos[half][:], vv[j][:, h * 65:(h + 1) * 65], e[:, esl],
                    start=(j == 0), stop=(j == NT - 1))

        def norm(h, psos):
            """Normalize O'^T by its rowsum into ot (DVE + DRAM-bounce
            partition broadcast; see baseline docstring)."""
            off = (h % 2) * 64
            for half in range(2):
                sl = bass.ts(half, 512)
                pso = psos[half]
                o_sb = small.tile([64, 512], bf16, tag="osb2", name=f"o_sb{h}_{half}")
                nc.vector.tensor_copy(o_sb[:], pso[0:64, :])
                srow = small.tile([1, 512], f32, tag="srow", name=f"srow{h}_{half}")
                nc.vector.tensor_copy(srow[:], pso[64:65, :])
                r1 = small.tile([1, 512], f32, tag="rc", name=f"rc{h}_{half}")
                nc.vector.reciprocal_approx_fast(out=r1[:], in_=srow[:])
                r1b = small.tile([1, 512], bf16, tag="rcb", name=f"rcb{h}_{half}")
                nc.vector.tensor_copy(r1b[:], r1[:])
                scr = drp.tile([1, 512], bf16, tag="scr", name=f"scr{h}_{half}")
                nc.gpsimd.dma_start(scr[:], r1b[:])
                s = scr[:]
                src_b = bass.AP(tensor=s.tensor, offset=s.offset,
                                ap=[[0, 64]] + list(s.ap[1:]))
                rbc = small.tile([64, 512], bf16, tag="rbc", name=f"rbc{h}_{half}")
                nc.gpsimd.dma_start(rbc[:], src_b)
                nc.vector.tensor_mul(ot[h // 2][off:off + 64, sl], o_sb[:], rbc[:])

        ones64 = persist.tile([1, 64], bf16, tag="ones64", name="ones64")
        nc.gpsimd.memset(ones64[:], 1.0)

        def norm_tail(h, psos, bpool, btag):
            """Tail-pair norm: partition-broadcast via a K=1 PE matmul instead
            of the DRAM bounce — shorter critical chain for the last heads.
            The o_sb/srow copies release the psos slots BEFORE pbc allocates
            from the same pool (deadlock-free); the final mul reads the
            broadcast directly from PSUM (one PSUM operand is legal)."""
            off = (h % 2) * 64
            for half in range(2):
                sl = bass.ts(half, 512)
                pso = psos[half]
                o_sb = small.tile([64, 512], bf16, tag="osb2", name=f"t_osb{h}_{half}")
                nc.vector.tensor_copy(o_sb[:], pso[0:64, :])
                srow = small.tile([1, 512], f32, tag="srow", name=f"tsrow{h}_{half}")
                nc.vector.tensor_copy(srow[:], pso[64:65, :])
                r1 = small.tile([1, 512], f32, tag="rc", name=f"trc{h}_{half}")
                nc.vector.reciprocal_approx_fast(out=r1[:], in_=srow[:])
                r1b = small.tile([1, 512], bf16, tag="rcb", name=f"trcb{h}_{half}")
                nc.scalar.copy(r1b[:], r1[:])
                pbc = bpool.tile([64, 512], f32, tag=btag, name=f"tpbc{h}_{half}")
                nc.tensor.matmul(pbc[:], ones64[:], r1b[:], start=True, stop=True)
                nc.vector.tensor_mul(ot[h // 2][off:off + 64, sl], o_sb[:], pbc[:])

        def pv_units(pm, tail=False):
            """PV + norm of pair pm as a unit list (consumed in pair pm+1).
            tail=True: PV-B runs on pmm (concurrent with PV-A on po, no slot
            serialization) and norms use the PE-broadcast variant."""
            st = {}
            units = []

            def mkA():
                st['A'] = po_tiles(2 * pm)

            def mkB():
                if tail:
                    st['B'] = po_tiles(2 * pm + 1, pss, "s")
                else:
                    st['B'] = po_tiles(2 * pm + 1)

            for j in range(NT):
                def uA(j=j):
                    if 'A' not in st:
                        mkA()
                    pv_step(2 * pm, st['A'], j, es[(pm, j)][0])
                units.append(uA)
            if tail:
                units.append(lambda: norm_tail(2 * pm, st['A'], po, "o"))
            else:
                units.append(lambda: norm(2 * pm, st['A']))
            for j in range(NT):
                def uB(j=j):
                    if 'B' not in st:
                        mkB()
                    pv_step(2 * pm + 1, st['B'], j, es[(pm, j)][1])
                units.append(uB)
            if tail:
                units.append(lambda: norm_tail(2 * pm + 1, st['B'], pss, "s"))
            else:
                units.append(lambda: norm(2 * pm + 1, st['B']))
            return units

        # ---- prologue: q0 (pmm), k0 (pss) only, paced by DMA arrivals —
        # q1/k1 become pair-0 fillers so the first scores land sooner.
        b_q0s, b_q0f = qk_builder(0, pmm, "mm")
        b_k0s, b_k0f = qk_builder(8, pss, "s")
        for kt in range(KT):
            b_q0s(kt)
            b_k0s(kt)
        b_q0f()
        b_k0f()

        # ---- builder assignment per pair (fillers) ----
        def QK(j, pool, tag):
            return lambda: builder_units(lambda: qk_builder(j, pool, tag))

        def VB(nt, pool, tag):
            return lambda: builder_units(lambda: v_builder(nt, pool, tag))

        # NOTE: every v builder must be EMITTED in pair 0 — Tile's dependency
        # tracking is program-order-based, so a PV read of vv[j] emitted
        # before the v builder's writes would silently miss the dependency.
        builders_by_pair = [
            [QK(1, po, "o"), QK(9, po, "o"), VB(0, pmm, "mm"), VB(1, po, "o"),
             VB(2, pmm, "mm"), VB(3, po, "o"), VB(4, pmm, "mm"),
             VB(5, po, "o"), VB(6, pmm, "mm"), VB(7, po, "o")],
            [QK(2, pmm, "mm"), QK(10, pmm, "mm")],
            [QK(3, pmm, "mm"), QK(11, pmm, "mm")],
            [QK(4, pmm, "mm"), QK(12, pmm, "mm")],
            [QK(5, pmm, "mm"), QK(13, pmm, "mm")],
            [QK(6, pmm, "mm"), QK(14, pmm, "mm")],
            [QK(7, pmm, "mm"), QK(15, pmm, "mm")],
            [],  # pair 7: proj pre-accumulation, set up below
        ]

        def proj_tile(nt, pool):
            """proj output row-tile nt: final[nq, co] = sum_kt ot[kt].T @ wp[kt].
            pss pool: one [128,1024] tile (both banks); else two [128,512]."""
            st = {}
            ntsl = slice(nt * 128, (nt + 1) * 128)

            def step(kt):
                if 'ph' not in st:
                    if pool is pss:
                        t = pss.tile([128, N], f32, tag="s", name=f"ps_pj{nt}")
                        st['ph'] = [t[:, 0:512], t[:, 512:1024]]
                    else:
                        tg = "mm" if pool is pmm else "o"
                        st['ph'] = [pool.tile([128, 512], f32, tag=tg,
                                              name=f"ps_pj{nt}_{x}")[:]
                                    for x in range(2)]
                for half in range(2):
                    sl = bass.ts(half, 512)
                    nc.tensor.matmul(
                        st['ph'][half], ot[kt][:, ntsl], wp[kt][:, sl],
                        start=(kt == 0), stop=(kt == KT - 1 and not has_bias))
                    if has_bias and kt == KT - 1:
                        nc.tensor.matmul(
                            st['ph'][half], xones[:, ntsl], wpb[:, sl],
                            start=False, stop=True)

            def finish():
                osb = outp.tile([128, N], f32, tag="osb", name=f"osb{nt}")
                for half in range(2):
                    nc.vector.tensor_copy(osb[:, bass.ts(half, 512)], st['ph'][half])
                nc.sync.dma_start(out_d[ntsl, :], osb[:])

            return step, finish

        pj1 = proj_tile(1, pmm)

        # ---- main pipeline: 8 pairs x 8 slots ----
        for p in range(8):
            # expand builder units for this pair
            units = []
            for mk in builders_by_pair[p]:
                units.extend(mk())
            if p == 7:
                # proj pre-accumulation on pmm (free: no builders this pair),
                # kt<=5 only: ot[6] writes (norm of pair 6) are emitted inside
                # THIS pair's pv units — a kt=6 read emitted here would
                # precede them (missed dependency).
                units.extend([(lambda k=kt: pj1[0](k)) for kt in range(6)])
            pvs = pv_units(p - 1) if p > 0 else []
            u_sched = _spread(units, NT) if units else [[] for _ in range(NT)]
            pv_sched = _spread(pvs, NT) if pvs else [[] for _ in range(NT)]
            for j in range(NT):
                if p == 0:
                    # pair 0: scores first — no pv fillers exist and the
                    # builder units are DMA-gated; emitting scores last would
                    # queue them behind ~20 priority-earlier filler MMs and
                    # delay the very first exp by ~5us.
                    scores(p, j)
                    for u in u_sched[j]:
                        u()
                    continue
                if p == 7:
                    # pair 7: ACT is the constraint on when the last exp
                    # lands (PE has slack — no builders), and the whole
                    # epilogue chains off it. Scores right after the pv
                    # units (fully-first would stall the in-order queue).
                    for u in pv_sched[j]:
                        u()
                    scores(p, j)
                    for u in u_sched[j]:
                        u()
                    continue
                for u in pv_sched[j]:
                    u()
                for u in u_sched[j]:
                    u()
                scores(p, j)

        # ---- epilogue ----
        # PV(7) with PV-A on po, PV-B on pmm (concurrent), PE-broadcast norms.
        for u in pv_units(7, tail=True):
            u()
        if DEBUG_DUMPS:
            for kt in range(KT):
                nc.sync.dma_start(dbg_ot[kt], ot[kt][:])

        # kt=6 step for the pre-accumulated tile (norm(6,B) now emitted)
        pj1[0](6)
        # pj2 rides po the moment norm(7,A)'s copies release it — overlaps
        # with norm(7,B)'s chain on DVE.
        pj2 = proj_tile(2, po)
        for kt in range(7):
            pj2[0](kt)
        # close the pre-accumulated tiles (kt=7 is after norm(7,B) above)
        for pj in (pj1, pj2):
            pj[0](7)
            pj[1]()
        # remaining tiles cascade through the freed pools
        for nt, pool in ((5, pss), (6, pss), (3, pmm), (4, po), (7, pss), (0, pss)):
            s, f = proj_tile(nt, pool)
            for kt in range(KT):
                s(kt)
            f()


def _get_compiled(has_bias):
    key = ("nc", has_bias)
    if key in _CACHE:
        return _CACHE[key]
    import concourse.bass as bass
    import concourse.mybir as mybir
    from concourse import bacc, tile

    nc = bacc.Bacc("TRN2", target_bir_lowering=False, debug=False, num_devices=B)
    with tile.TileContext(nc) as tc:
        _build_graph(nc, tc, bass, mybir, has_bias)
    nc.compile()
    _CACHE[key] = nc
    return nc


def _in_maps(x, w_qkv, b_qkv, w_proj, b_proj):
    xT = np.ascontiguousarray(np.transpose(np.asarray(x, np.float32), (0, 2, 1))).astype(BF16)
    ones = np.ones((1, N), BF16)
    wq = np.concatenate([np.asarray(w_qkv, np.float32).T,
                         np.asarray(b_qkv, np.float32)[None, :]], 0).astype(BF16)
    wp = np.concatenate([np.asarray(w_proj, np.float32).T,
                         np.asarray(b_proj, np.float32)[None, :]], 0).astype(BF16)
    wq = np.ascontiguousarray(wq)
    wp = np.ascontiguousarray(wp)
    return [
        {"xT": np.ascontiguousarray(np.concatenate([xT[b], ones], 0)),
         "wqkvT": wq, "wprojT": wp}
        for b in range(B)
    ]


def _ensure_ntff_hook():
    """The agent image's `antenv` lacks `axon_hooks`; provide the registry
    module + ctypes hook so neuron-profile NTFF capture works when tracing."""
    import importlib
    import types

    try:
        importlib.import_module("antenv.axon_hooks")
        return
    except ImportError:
        pass
    mod = types.ModuleType("antenv.axon_hooks")
    mod._hook = None

    def set_axon_ntff_profile_hook(h):
        mod._hook = h

    def get_axon_ntff_profile_hook():
        return mod._hook

    mod.set_axon_ntff_profile_hook = set_axon_ntff_profile_hook
    mod.get_axon_ntff_profile_hook = get_axon_ntff_profile_hook
    import antenv

    antenv.axon_hooks = mod
    sys.modules["antenv.axon_hooks"] = mod
    try:
        from trn_agent_boot.trn_boot import _ntff_profile_via_ctypes

        hook = _ntff_profile_via_ctypes("/opt/axon/libaxon_pjrt.so")
        if hook is not None:
            mod._hook = hook
    except Exception:
        pass


def kernel(x, w_qkv, b_qkv, w_proj, b_proj):
    global LAST_RESULTS
    import os

    if os.environ.get("BASS_TRACE"):
        _ensure_ntff_hook()
    from concourse.bass_utils import run_bass_kernel_spmd

    has_bias = bool(np.any(np.asarray(b_qkv)) or np.any(np.asarray(b_proj)))
    nc = _get_compiled(has_bias)
    maps = _in_maps(x, w_qkv, b_qkv, w_proj, b_proj)
    res = run_bass_kernel_spmd(nc, maps, core_ids=list(range(B)))
    LAST_RESULTS = res
    return np.stack([res.results[b]["out"] for b in range(B)]).astype(np.float32)


# revision 45
# speedup vs baseline: 1.0067x; 1.0060x over previous
"""Trainium2 Bass kernel for multi-head attention (B=8, N=1024, C=1024, H=16).

Sharding: pure data parallel - one batch element per NeuronCore (8 cores),
no collectives. Host pre-transposes/casts weights and activations to bf16;
all matmuls run bf16 with fp32 PSUM accumulation.

v2 schedule: one uniform software pipeline instead of serial phases.
  - Input DMA is column-sliced so the q0/k0/q1/k1 weight slices land first;
    scores for pair 0 start ~20us earlier than with full-row weight loads.
  - 32 dummy warmup matmuls run during the DMA lead-in so the PE HAM clock
    gate is already at 2.4 GHz when real work arrives.
  - Pair p's PV + normalization run as PE filler inside pair p+1's score
    slots, so ACT (exp) never sees a pair-boundary bubble and the old 22us
    "build all v tiles" hole is gone (v tiles are fillers too).
  - Within each slot, fillers are emitted BEFORE the 4 score matmuls, so
    both score PSUM slots are free by the time the score MMs issue and the
    two heads' K=64 matmuls actually run concurrently on row groups 0/64.
  - proj pre-accumulates one tile during pair 7, rest in the epilogue on
    the freed score PSUM slots, output DMAs overlapped per row tile.
"""

import sys

import numpy as np

if "/opt/trn_rl_repo" not in sys.path:
    sys.path.insert(0, "/opt/trn_rl_repo")

import ml_dtypes

BF16 = ml_dtypes.bfloat16

C = 1024          # model dim
N = 1024          # sequence length
H = 16            # heads
D = 64            # head dim
B = 8             # batch == number of cores
KT = C // 128     # 8 contraction tiles
NT = N // 128     # 8 sequence tiles
SCALE = float(D) ** -0.5

_CACHE = {}
LAST_RESULTS = None
DEBUG_DUMPS = False


def _spread(lst, nslots):
    """Distribute list into nslots chunks, preserving order."""
    out = [[] for _ in range(nslots)]
    n = len(lst)
    for i, x in enumerate(lst):
        out[i * nslots // n].append(x)
    return out


def _build_graph(nc, tc, bass, mybir, has_bias):
    from contextlib import ExitStack

    f32 = mybir.dt.float32
    bf16 = mybir.dt.bfloat16
    Exp = mybir.ActivationFunctionType.Exp

    xT_d = nc.dram_tensor("xT", [C + 1, N], bf16, kind="ExternalInput").ap()
    wq_d = nc.dram_tensor("wqkvT", [C + 1, 3 * C], bf16, kind="ExternalInput").ap()
    wp_d = nc.dram_tensor("wprojT", [C + 1, C], bf16, kind="ExternalInput").ap()
    out_d = nc.dram_tensor("out", [N, C], f32, kind="ExternalOutput").ap()
    if DEBUG_DUMPS:
        dbg_qk = nc.dram_tensor("dbg_qk", [16, 128, N], bf16, kind="ExternalOutput").ap()
        dbg_vv = nc.dram_tensor("dbg_vv", [NT, 128, H * 65], bf16, kind="ExternalOutput").ap()
        dbg_es = nc.dram_tensor("dbg_es", [NT, 2, 128, N], bf16, kind="ExternalOutput").ap()
        dbg_ot = nc.dram_tensor("dbg_ot", [KT, 128, N], bf16, kind="ExternalOutput").ap()

    with ExitStack() as ctx:
        persist = ctx.enter_context(tc.tile_pool(name="persist", bufs=1))
        qkp = ctx.enter_context(tc.tile_pool(name="qkp", bufs=5))
        expp = ctx.enter_context(tc.tile_pool(name="expp", bufs=23))
        small = ctx.enter_context(tc.tile_pool(name="small", bufs=4))
        outp = ctx.enter_context(tc.tile_pool(name="outp", bufs=2))
        # PSUM budget 8 banks: pmm 2x[128,512] (2) + pss 2x[128,1024] (4)
        # + po 2x[128,512]-sized (2).
        pmm = ctx.enter_context(tc.tile_pool(name="pmm", bufs=2, space="PSUM"))
        pss = ctx.enter_context(tc.tile_pool(name="pss", bufs=2, space="PSUM"))
        po = ctx.enter_context(tc.tile_pool(name="po", bufs=2, space="PSUM"))
        drp = ctx.enter_context(tc.tile_pool(name="drp", bufs=2, space="DRAM"))

        # ---- persistent SBUF tensors ----
        xt = [persist.tile([128, N], bf16, tag=f"xt{i}", name=f"xt{i}") for i in range(KT)]
        wq = [persist.tile([128, 3 * C], bf16, tag=f"wq{i}", name=f"wq{i}") for i in range(KT)]
        wp = [persist.tile([128, C], bf16, tag=f"wp{i}", name=f"wp{i}") for i in range(KT)]
        vv = [persist.tile([128, H * 65], bf16, tag=f"vv{i}", name=f"vv{i}") for i in range(NT)]
        ot = [persist.tile([128, N], bf16, tag=f"ot{i}", name=f"ot{i}") for i in range(KT)]
        scrw = persist.tile([128, 512], bf16, tag="scrw", name="scrw")
        if has_bias:
            xones = persist.tile([1, N], bf16, tag="xones", name="xones")
            wqb = persist.tile([1, 3 * C], bf16, tag="wqb", name="wqb")
            wpb = persist.tile([1, C], bf16, tag="wpb", name="wpb")

        # ---- PE warmup: dummy matmuls during the DMA lead-in keep the HAM
        # clock gate at 2.4GHz so the first real matmuls aren't half-rate.
        nc.vector.memset(scrw[:], 0.0)
        pwarm = po.tile([128, 512], f32, tag="o", name="pwarm")
        for _ in range(16):
            nc.tensor.matmul(pwarm[:], scrw[:, 0:128], scrw[:], start=True, stop=True)
        wdrain = small.tile([1, 16], f32, tag="wdrain", name="wdrain")
        nc.vector.tensor_copy(wdrain[:], pwarm[0:1, 0:16])

        # preload the Exp activation table during the DMA phase.
        warm = small.tile([1, 16], f32, tag="warm", name="warm")
        nc.vector.memset(warm[:], 0.0)
        nc.scalar.activation(warm[:], warm[:], Exp, scale=1.0)

        # ---- input DMAs, column-sliced by first use ----
        def eng(i):
            return nc.sync if i % 2 == 0 else nc.gpsimd

        def wq_slice(kt, c0, c1, e):
            e.dma_start(wq[kt][:, c0:c1], wq_d[kt * 128:(kt + 1) * 128, c0:c1])

        # critical prefix: xt + q0,k0,q1,k1 column slices, kt-major
        USE_SLICED = True
        if USE_SLICED:
            for kt in range(KT):
                e = eng(kt)
                e.dma_start(xt[kt][:], xT_d[kt * 128:(kt + 1) * 128, :])
                for t in (0, 8):
                    wq_slice(kt, t * 128, (t + 1) * 128, e)
            # q1/k1 (first pair-0 builders in the static schedule), then v
            for kt in range(KT):
                e = eng(kt)
                for t in (1, 9):
                    wq_slice(kt, t * 128, (t + 1) * 128, e)
            for kt in range(KT):
                wq_slice(kt, 2 * C, 3 * C, eng(kt + 1))
            # q rest, k rest
            for kt in range(KT):
                wq_slice(kt, 2 * 128, C, eng(kt))
                wq_slice(kt, C + 2 * 128, 2 * C, eng(kt))
        else:
            for kt in range(KT):
                e = eng(kt)
                e.dma_start(xt[kt][:], xT_d[kt * 128:(kt + 1) * 128, :])
                e.dma_start(wq[kt][:], wq_d[kt * 128:(kt + 1) * 128, :])
        # proj weights last
        for kt in range(KT):
            eng(kt + 1).dma_start(wp[kt][:], wp_d[kt * 128:(kt + 1) * 128, :])
        if has_bias:
            nc.sync.dma_start(xones[:], xT_d[C:C + 1, :])
            nc.sync.dma_start(wqb[:], wq_d[C:C + 1, :])
            nc.sync.dma_start(wpb[:], wp_d[C:C + 1, :])

        qk = {}   # qk-tile index (0..7 q, 8..15 k) -> sbuf tile
        es = {}   # (pair, j) -> (eA, eB)

        def qk_builder(j_tile, pool, tag):
            """Incremental qk tile: qkT[o, n] = w_qkvT[:, o].T @ xT."""
            t = qkp.tile([128, N], bf16, tag="qk", name=f"qk{j_tile}")
            ph = [pool.tile([128, 512], f32, tag=tag, name=f"ps_qk{j_tile}_{x}")
                  for x in range(2)]

            def step(kt):
                for half in range(2):
                    sl = bass.ts(half, 512)
                    nc.tensor.matmul(
                        ph[half][:], wq[kt][:, j_tile * 128:(j_tile + 1) * 128],
                        xt[kt][:, sl],
                        start=(kt == 0), stop=(kt == KT - 1 and not has_bias))
                    if has_bias and kt == KT - 1:
                        nc.tensor.matmul(
                            ph[half][:], wqb[:, j_tile * 128:(j_tile + 1) * 128],
                            xones[:, sl], start=False, stop=True)

            def finish():
                for half in range(2):
                    nc.vector.tensor_copy(t[:, bass.ts(half, 512)], ph[half][:])
                qk[j_tile] = t
                if DEBUG_DUMPS:
                    nc.sync.dma_start(dbg_qk[j_tile], t[:])

            return step, finish

        def v_builder(nt, pool, tag):
            """Incremental v tile: v[n, o] = xT[:, n].T @ w_qkvT[:, 2C:].
            Stored with stride-65 head blocks; col 64 = ones (rowsum trick)."""
            dst = vv[nt][:].rearrange("p (h w) -> p h w", w=65)
            phs = [pool.tile([128, 512], f32, tag=tag, name=f"ps_v{nt}_{x}")
                   for x in range(2)]

            def step(kt):
                for half in range(2):
                    sl = bass.ds(2 * C + half * 512, 512)
                    nc.tensor.matmul(
                        phs[half][:], xt[kt][:, nt * 128:(nt + 1) * 128],
                        wq[kt][:, sl],
                        start=(kt == 0), stop=(kt == KT - 1 and not has_bias))
                    if has_bias and kt == KT - 1:
                        nc.tensor.matmul(
                            phs[half][:], xones[:, nt * 128:(nt + 1) * 128],
                            wqb[:, sl], start=False, stop=True)

            def finish():
                for half in range(2):
                    nc.vector.tensor_copy(
                        dst[:, half * 8:(half + 1) * 8, 0:64],
                        phs[half][:].rearrange("p (h w) -> p h w", w=64))
                nc.gpsimd.memset(dst[:, :, 64:65], 1.0)
                if DEBUG_DUMPS:
                    nc.sync.dma_start(dbg_vv[nt], vv[nt][:])

            return step, finish

        def builder_units(mk):
            """Expand a builder into a list of emit-closures (8 steps + finish)."""
            step, fin = mk()
            return [(lambda s=step, k=kt: s(k)) for kt in range(KT)] + [fin]

        def scores(p, j):
            """Emit pair-p scores for nk-tile j + 2 exps. Each head's matmul
            is split into two M=64 halves so the 4 matmuls per nq-half occupy
            DISJOINT 64x64 quadrant sets (rows 0/64 x cols 0/64) and run
            4-way concurrent on the PE (span ~= one matmul). Emitted LAST in
            each slot so both pss slots are already free when they issue."""
            qA = qk[p][0:64, :]
            kA = qk[8 + p][0:64, :]
            qB = qk[p][64:128, :]
            kB = qk[8 + p][64:128, :]
            jsl = slice(j * 128, (j + 1) * 128)
            psA = pss.tile([128, N], f32, tag="s", name=f"s{p}_{j}a")
            psB = pss.tile([128, N], f32, tag="s", name=f"s{p}_{j}b")
            nc.tensor.matmul(psA[:, 0:512], kA[:, jsl], qA[:, 0:512],
                             start=True, stop=True)
            nc.tensor.matmul(psB[:, 0:512], kB[:, jsl], qB[:, 0:512],
                             start=True, stop=True)
            nc.tensor.matmul(psA[:, 512:1024], kA[:, jsl], qA[:, 512:1024],
                             start=True, stop=True)
            eA = expp.tile([128, N], bf16, tag="es", name=f"e{p}_{j}a")
            nc.scalar.activation(eA[:], psA[:], Exp, scale=SCALE)
            nc.tensor.matmul(psB[:, 512:1024], kB[:, jsl], qB[:, 512:1024],
                             start=True, stop=True)
            eB = expp.tile([128, N], bf16, tag="es", name=f"e{p}_{j}b")
            nc.scalar.activation(eB[:], psB[:], Exp, scale=SCALE)
            es[(p, j)] = (eA, eB)
            if DEBUG_DUMPS and p == 0:
                nc.sync.dma_start(dbg_es[j, 0], eA[:])
                nc.sync.dma_start(dbg_es[j, 1], eB[:])

        def po_tiles(h, pool=None, tag=None):
            pool = pool or po
            tag = tag or "o"
            return [pool.tile([65, 512], f32, tag=tag, name=f"pso{h}_{x}")
                    for x in range(2)]

        def pv_step(h, psos, j, e):
            """One nk-tile of [O'^T ; rowsum] accumulation (both nq halves)."""
            for half in range(2):
                esl = bass.ts(half, 512)
                nc.tensor.matmul(
                    psos[half][:], vv[j][:, h * 65:(h + 1) * 65], e[:, esl],
                    start=(j == 0), stop=(j == NT - 1))

        def norm(h, psos):
            """Normalize O'^T by its rowsum into ot (DVE + DRAM-bounce
            partition broadcast; see baseline docstring)."""
            off = (h % 2) * 64
            for half in range(2):
                sl = bass.ts(half, 512)
                pso = psos[half]
                o_sb = small.tile([64, 512], bf16, tag="osb2", name=f"o_sb{h}_{half}")
                nc.vector.tensor_copy(o_sb[:], pso[0:64, :])
                srow = small.tile([1, 512], f32, tag="srow", name=f"srow{h}_{half}")
                nc.vector.tensor_copy(srow[:], pso[64:65, :])
                r1 = small.tile([1, 512], f32, tag="rc", name=f"rc{h}_{half}")
                nc.vector.reciprocal_approx_fast(out=r1[:], in_=srow[:])
                r1b = small.tile([1, 512], bf16, tag="rcb", name=f"rcb{h}_{half}")
                nc.vector.tensor_copy(r1b[:], r1[:])
                scr = drp.tile([1, 512], bf16, tag="scr", name=f"scr{h}_{half}")
                nc.gpsimd.dma_start(scr[:], r1b[:])
                s = scr[:]
                src_b = bass.AP(tensor=s.tensor, offset=s.offset,
                                ap=[[0, 64]] + list(s.ap[1:]))
                rbc = small.tile([64, 512], bf16, tag="rbc", name=f"rbc{h}_{half}")
                nc.gpsimd.dma_start(rbc[:], src_b)
                nc.vector.tensor_mul(ot[h // 2][off:off + 64, sl], o_sb[:], rbc[:])

        ones64 = persist.tile([1, 64], bf16, tag="ones64", name="ones64")
        nc.gpsimd.memset(ones64[:], 1.0)

        def norm_tail(h, psos, bpool, btag):
            """Tail-pair norm: partition-broadcast via a K=1 PE matmul instead
            of the DRAM bounce — shorter critical chain for the last heads.
            The o_sb/srow copies release the psos slots BEFORE pbc allocates
            from the same pool (deadlock-free); the final mul reads the
            broadcast directly from PSUM (one PSUM operand is legal)."""
            off = (h % 2) * 64
            for half in range(2):
                sl = bass.ts(half, 512)
                pso = psos[half]
                o_sb = small.tile([64, 512], bf16, tag="osb2", name=f"t_osb{h}_{half}")
                nc.vector.tensor_copy(o_sb[:], pso[0:64, :])
                srow = small.tile([1, 512], f32, tag="srow", name=f"tsrow{h}_{half}")
                nc.vector.tensor_copy(srow[:], pso[64:65, :])
                r1 = small.tile([1, 512], f32, tag="rc", name=f"trc{h}_{half}")
                nc.vector.reciprocal_approx_fast(out=r1[:], in_=srow[:])
                r1b = small.tile([1, 512], bf16, tag="rcb", name=f"trcb{h}_{half}")
                nc.scalar.copy(r1b[:], r1[:])
                pbc = bpool.tile([64, 512], f32, tag=btag, name=f"tpbc{h}_{half}")
                nc.tensor.matmul(pbc[:], ones64[:], r1b[:], start=True, stop=True)
                nc.vector.tensor_mul(ot[h // 2][off:off + 64, sl], o_sb[:], pbc[:])

        def pv_units(pm, tail=False):
            """PV + norm of pair pm as a unit list (consumed in pair pm+1).
            tail=True: PV-B runs on pmm (concurrent with PV-A on po, no slot
            serialization) and norms use the PE-broadcast variant."""
            st = {}
            units = []

            def mkA():
                st['A'] = po_tiles(2 * pm)

            def mkB():
                if tail:
                    st['B'] = po_tiles(2 * pm + 1, pss, "s")
                else:
                    st['B'] = po_tiles(2 * pm + 1)

            for j in range(NT):
                def uA(j=j):
                    if 'A' not in st:
                        mkA()
                    pv_step(2 * pm, st['A'], j, es[(pm, j)][0])
                units.append(uA)
            if tail:
                units.append(lambda: norm_tail(2 * pm, st['A'], po, "o"))
            else:
                units.append(lambda: norm(2 * pm, st['A']))
            for j in range(NT):
                def uB(j=j):
                    if 'B' not in st:
                        mkB()
                    pv_step(2 * pm + 1, st['B'], j, es[(pm, j)][1])
                units.append(uB)
            if tail:
                units.append(lambda: norm_tail(2 * pm + 1, st['B'], pss, "s"))
            else:
                units.append(lambda: norm(2 * pm + 1, st['B']))
            return units

        # ---- prologue: q0 (pmm), k0 (pss) only, paced by DMA arrivals —
        # q1/k1 become pair-0 fillers so the first scores land sooner.
        b_q0s, b_q0f = qk_builder(0, pmm, "mm")
        b_k0s, b_k0f = qk_builder(8, pss, "s")
        for kt in range(KT):
            b_q0s(kt)
            b_k0s(kt)
        b_q0f()
        b_k0f()

        # ---- builder assignment per pair (fillers) ----
        def QK(j, pool, tag):
            return lambda: builder_units(lambda: qk_builder(j, pool, tag))

        def VB(nt, pool, tag):
            return lambda: builder_units(lambda: v_builder(nt, pool, tag))

        # NOTE: every v builder must be EMITTED in pair 0 — Tile's dependency
        # tracking is program-order-based, so a PV read of vv[j] emitted
        # before the v builder's writes would silently miss the dependency.
        builders_by_pair = [
            [QK(1, po, "o"), QK(9, po, "o"), VB(0, pmm, "mm"), VB(1, po, "o"),
             VB(2, pmm, "mm"), VB(3, po, "o"), VB(4, pmm, "mm"),
             VB(5, po, "o"), VB(6, pmm, "mm"), VB(7, po, "o")],
            [QK(2, pmm, "mm"), QK(10, pmm, "mm")],
            [QK(3, pmm, "mm"), QK(11, pmm, "mm")],
            [QK(4, pmm, "mm"), QK(12, pmm, "mm")],
            [QK(5, pmm, "mm"), QK(13, pmm, "mm")],
            [QK(6, pmm, "mm"), QK(14, pmm, "mm")],
            [QK(7, pmm, "mm"), QK(15, pmm, "mm")],
            [],  # pair 7: proj pre-accumulation, set up below
        ]

        def proj_tile(nt, pool):
            """proj output row-tile nt: final[nq, co] = sum_kt ot[kt].T @ wp[kt].
            pss pool: one [128,1024] tile (both banks); else two [128,512]."""
            st = {}
            ntsl = slice(nt * 128, (nt + 1) * 128)

            def step(kt):
                if 'ph' not in st:
                    if pool is pss:
                        t = pss.tile([128, N], f32, tag="s", name=f"ps_pj{nt}")
                        st['ph'] = [t[:, 0:512], t[:, 512:1024]]
                    else:
                        tg = "mm" if pool is pmm else "o"
                        st['ph'] = [pool.tile([128, 512], f32, tag=tg,
                                              name=f"ps_pj{nt}_{x}")[:]
                                    for x in range(2)]
                for half in range(2):
                    sl = bass.ts(half, 512)
                    nc.tensor.matmul(
                        st['ph'][half], ot[kt][:, ntsl], wp[kt][:, sl],
                        start=(kt == 0), stop=(kt == KT - 1 and not has_bias))
                    if has_bias and kt == KT - 1:
                        nc.tensor.matmul(
                            st['ph'][half], xones[:, ntsl], wpb[:, sl],
                            start=False, stop=True)

            def finish():
                osb = outp.tile([128, N], f32, tag="osb", name=f"osb{nt}")
                for half in range(2):
                    nc.vector.tensor_copy(osb[:, bass.ts(half, 512)], st['ph'][half])
                nc.sync.dma_start(out_d[ntsl, :], osb[:])

            return step, finish

        pj1 = proj_tile(1, pmm)

        # ---- main pipeline: 8 pairs x 8 slots ----
        for p in range(8):
            # expand builder units for this pair
            units = []
            for mk in builders_by_pair[p]:
                units.extend(mk())
            if p == 7:
                # proj pre-accumulation on pmm (free: no builders this pair),
                # kt<=5 only: ot[6] writes (norm of pair 6) are emitted inside
                # THIS pair's pv units — a kt=6 read emitted here would
                # precede them (missed dependency).
                units.extend([(lambda k=kt: pj1[0](k)) for kt in range(6)])
            pvs = pv_units(p - 1) if p > 0 else []
            u_sched = _spread(units, NT) if units else [[] for _ in range(NT)]
            pv_sched = _spread(pvs, NT) if pvs else [[] for _ in range(NT)]
            for j in range(NT):
                if p == 0:
                    # pair 0: scores first — no pv fillers exist and the
                    # builder units are DMA-gated; emitting scores last would
                    # queue them behind ~20 priority-earlier filler MMs and
                    # delay the very first exp by ~5us.
                    scores(p, j)
                    for u in u_sched[j]:
                        u()
                    continue

                for u in pv_sched[j]:
                    u()
                for u in u_sched[j]:
                    u()
                scores(p, j)

        # ---- epilogue ----
        # PV(7) with PV-A on po, PV-B on pmm (concurrent), PE-broadcast norms.
        for u in pv_units(7, tail=True):
            u()
        if DEBUG_DUMPS:
            for kt in range(KT):
                nc.sync.dma_start(dbg_ot[kt], ot[kt][:])

        # kt=6 step for the pre-accumulated tile (norm(6,B) now emitted)
        pj1[0](6)
        # pj2 rides po the moment norm(7,A)'s copies release it — overlaps
        # with norm(7,B)'s chain on DVE.
        pj2 = proj_tile(2, po)
        for kt in range(7):
            pj2[0](kt)
        # close the pre-accumulated tiles (kt=7 is after norm(7,B) above)
        for pj in (pj1, pj2):
            pj[0](7)
            pj[1]()
        # remaining tiles cascade through the freed pools
        for nt, pool in ((5, pss), (6, pss), (3, pmm), (4, po), (7, pss), (0, pss)):
            s, f = proj_tile(nt, pool)
            for kt in range(KT):
                s(kt)
            f()


def _get_compiled(has_bias):
    key = ("nc", has_bias)
    if key in _CACHE:
        return _CACHE[key]
    import concourse.bass as bass
    import concourse.mybir as mybir
    from concourse import bacc, tile

    nc = bacc.Bacc("TRN2", target_bir_lowering=False, debug=False, num_devices=B)
    with tile.TileContext(nc) as tc:
        _build_graph(nc, tc, bass, mybir, has_bias)
    nc.compile()
    _CACHE[key] = nc
    return nc


def _in_maps(x, w_qkv, b_qkv, w_proj, b_proj):
    xT = np.ascontiguousarray(np.transpose(np.asarray(x, np.float32), (0, 2, 1))).astype(BF16)
    ones = np.ones((1, N), BF16)
    wq = np.concatenate([np.asarray(w_qkv, np.float32).T,
                         np.asarray(b_qkv, np.float32)[None, :]], 0).astype(BF16)
    wp = np.concatenate([np.asarray(w_proj, np.float32).T,
                         np.asarray(b_proj, np.float32)[None, :]], 0).astype(BF16)
    wq = np.ascontiguousarray(wq)
    wp = np.ascontiguousarray(wp)
    return [
        {"xT": np.ascontiguousarray(np.concatenate([xT[b], ones], 0)),
         "wqkvT": wq, "wprojT": wp}
        for b in range(B)
    ]


def _ensure_ntff_hook():
    """The agent image's `antenv` lacks `axon_hooks`; provide the registry
    module + ctypes hook so neuron-profile NTFF capture works when tracing."""
    import importlib
    import types

    try:
        importlib.import_module("antenv.axon_hooks")
        return
    except ImportError:
        pass
    mod = types.ModuleType("antenv.axon_hooks")
    mod._hook = None

    def set_axon_ntff_profile_hook(h):
        mod._hook = h

    def get_axon_ntff_profile_hook():
        return mod._hook

    mod.set_axon_ntff_profile_hook = set_axon_ntff_profile_hook
    mod.get_axon_ntff_profile_hook = get_axon_ntff_profile_hook
    import antenv

    antenv.axon_hooks = mod
    sys.modules["antenv.axon_hooks"] = mod
    try:
        from trn_agent_boot.trn_boot import _ntff_profile_via_ctypes

        hook = _ntff_profile_via_ctypes("/opt/axon/libaxon_pjrt.so")
        if hook is not None:
            mod._hook = hook
    except Exception:
        pass


def kernel(x, w_qkv, b_qkv, w_proj, b_proj):
    global LAST_RESULTS
    import os

    if os.environ.get("BASS_TRACE"):
        _ensure_ntff_hook()
    from concourse.bass_utils import run_bass_kernel_spmd

    has_bias = bool(np.any(np.asarray(b_qkv)) or np.any(np.asarray(b_proj)))
    nc = _get_compiled(has_bias)
    maps = _in_maps(x, w_qkv, b_qkv, w_proj, b_proj)
    res = run_bass_kernel_spmd(nc, maps, core_ids=list(range(B)))
    LAST_RESULTS = res
    return np.stack([res.results[b]["out"] for b in range(B)]).astype(np.float32)
